# revision 1
# baseline (speedup 1.0000x reference)
"""Trainium2 kernel for nn_COSSIMMLP (gnn_message_passing).

reference semantics:
    src = prop_state[b, mask[...,0]]; dst = prop_state[b, mask[...,1]]
    vals = sigmoid(cossim(src, dst))          # [B, E]
    adj[b, i, j] = vals; adj[b, j, i] = vals  # dense [B, N, N]

Every scatter write at position (r, c) carries the identical value
sigmoid(cos(s_r, s_c)) (reversed edges / duplicate edges give bit-identical
f32 values in the reference), so the output is exactly

    adj = sigmoid(S_hat @ S_hat.T + Madd),  Madd = 0 at edge positions,
                                                   -240 elsewhere

with S_hat the eps-clamp-normalized rows.  sigmoid(x - 240) underflows to 0 in
f32, so non-edges are (numerically exact) zero.  The additive mask is an fp8
0/-240 matrix folded into the PE accumulation group via an identity matmul —
no vector-engine masking pass and only 1 byte/entry of mask DMA.  The mask
depends only on the integer index tensor, so the host precomputes it; all
float math (normalization, gram matmul, sigmoid) runs on device.

Sharding: 8 cores = 4 batches x 2 row-halves.  Each core computes a
[2048, 4096] slab of one batch's adjacency.  Per-core node order is rolled
by the row offset so that a single SPMD program (rows = local nodes 0..2047)
serves all cores; the host un-rolls output columns.
"""

import numpy as np
import ml_dtypes

B, N, D, E = 4, 4096, 256, 131072
NH = N // 2          # rows per core
P = 128              # partitions
NT = N // P          # 32 node tiles
MT = NH // P         # 16 row tiles per core
GRP = 8              # node tiles per normalization group
EPS = 1e-8
MASK_OFF = 0xF7      # fp8_e4m3 encoding of -240.0 (sigmoid underflows to exact 0f)

_prog = None


def _build_program():
    import concourse.tile as tile
    from concourse import bacc, mybir
    from concourse.masks import make_identity

    f32 = mybir.dt.float32
    f16 = mybir.dt.float16
    fp8 = mybir.dt.float8e4
    ACT = mybir.ActivationFunctionType
    ALU = mybir.AluOpType

    nc = bacc.Bacc("TRN2", target_bir_lowering=False, debug=False)
    s_in = nc.dram_tensor("s", [N, D], f32, kind="ExternalInput")
    m_in = nc.dram_tensor("m", [NH, N], fp8, kind="ExternalInput")
    out = nc.dram_tensor("out", [NH, N], f32, kind="ExternalOutput")

    with tile.TileContext(nc) as tc:
        with tc.tile_pool(name="const", bufs=1) as cpool:
            ident16 = cpool.tile([P, P], f16)
            make_identity(nc, ident16[:])
            ident8 = cpool.tile([P, P], fp8)
            make_identity(nc, ident8[:])
            # S_hat.T, split into the two 128-dim chunks of D=256
            st0 = cpool.tile([P, N], f16)
            st1 = cpool.tile([P, N], f16)

            # ---- phase A: load, normalize (per group of 8 node tiles), transpose
            with (
                tc.tile_pool(name="prep", bufs=1) as prep,
                tc.tile_pool(name="prep_g", bufs=2) as prep_g,
                tc.tile_pool(name="prep_sc", bufs=3) as prep_sc,
                tc.tile_pool(name="prep_ps", bufs=4, space="PSUM") as prep_ps,
            ):
                s_sb = prep.tile([P, NT, D], f32)
                shat = prep.tile([P, NT, D], f16)
                s_r = s_in.rearrange("(t p) d -> p t d", p=P)
                for grp in range(NT // GRP):
                    t0 = grp * GRP
                    nc.sync.dma_start(
                        out=s_sb[:, t0 : t0 + GRP, :], in_=s_r[:, t0 : t0 + GRP, :]
                    )
                    nsq = prep_g.tile([P, GRP], f32, tag="nsq")
                    for i in range(GRP):
                        sq = prep_sc.tile([P, D], f32, tag="sq")
                        nc.scalar.activation(
                            out=sq[:], in_=s_sb[:, t0 + i, :], func=ACT.Square,
                            accum_out=nsq[:, i : i + 1],
                        )
                    nrm = prep_g.tile([P, GRP], f32, tag="nrm")
                    nc.scalar.activation(out=nrm[:], in_=nsq[:], func=ACT.Sqrt)
                    nc.vector.tensor_scalar_max(out=nrm[:], in0=nrm[:], scalar1=EPS)
                    inv = prep_g.tile([P, GRP], f32, tag="inv")
                    nc.vector.reciprocal(out=inv[:], in_=nrm[:])
                    for i in range(GRP):
                        nc.vector.tensor_scalar_mul(
                            out=shat[:, t0 + i, :],
                            in0=s_sb[:, t0 + i, :],
                            scalar1=inv[:, i : i + 1],
                        )
                    for i in range(GRP):
                        t = t0 + i
                        for dch, std in ((0, st0), (1, st1)):
                            pt = prep_ps.tile([P, P], f16, tag="tp")
                            nc.tensor.transpose(
                                pt[:], shat[:, t, dch * P : (dch + 1) * P], ident16[:]
                            )
                            nc.vector.tensor_copy(
                                out=std[:, t * P : (t + 1) * P], in_=pt[:]
                            )

            # ---- phase B: gram matmul + fp8 mask add -> sigmoid -> store ----
            with (
                tc.tile_pool(name="mrow", bufs=8) as mrow,
                tc.tile_pool(name="outp", bufs=3) as outp,
                tc.tile_pool(name="mmps", bufs=2, space="PSUM") as mmps,
            ):
                for m in range(MT):
                    msk = mrow.tile([P, N], fp8, tag="msk")
                    nc.scalar.dma_start(out=msk[:], in_=m_in[m * P : (m + 1) * P, :])
                    ot = outp.tile([P, N], f32, tag="ot")
                    for g in range(2):
                        ps = mmps.tile([P, 2048], f32, tag="ps")
                        for k, stk in ((0, st0), (1, st1)):
                            lhsT = stk[:, m * P : (m + 1) * P]
                            for q in range(4):
                                nc.tensor.matmul(
                                    ps[:, q * 512 : (q + 1) * 512],
                                    lhsT=lhsT,
                                    rhs=stk[:, g * 2048 + q * 512 : g * 2048 + (q + 1) * 512],
                                    start=(k == 0),
                                    stop=False,
                                )
                        for q in range(4):
                            nc.tensor.matmul(
                                ps[:, q * 512 : (q + 1) * 512],
                                lhsT=ident8[:],
                                rhs=msk[:, g * 2048 + q * 512 : g * 2048 + (q + 1) * 512],
                                start=False,
                                stop=True,
                            )
                        nc.scalar.activation(
                            out=ot[:, g * 2048 : (g + 1) * 2048],
                            in_=ps[:],
                            func=ACT.Sigmoid,
                        )
                    nc.sync.dma_start(out=out[m * P : (m + 1) * P, :], in_=ot[:])

    nc.compile()
    return nc


def _host_prep(prop_state, mask):
    prop = np.ascontiguousarray(np.asarray(prop_state), dtype=np.float32)
    mk = np.asarray(mask)
    i = mk[..., 0].astype(np.int64)
    j = mk[..., 1].astype(np.int64)
    fp8np = ml_dtypes.float8_e4m3
    adjmask = np.full((B, N * N), MASK_OFF, dtype=np.uint8)
    for b in range(B):
        flat = adjmask[b]
        flat[i[b] * N + j[b]] = 0
        flat[j[b] * N + i[b]] = 0
    adjmask = adjmask.reshape(B, N, N)

    in_maps = []
    for c in range(8):
        b, h = divmod(c, 2)
        r = h * NH
        s_roll = prop[b] if r == 0 else np.roll(prop[b], -r, axis=0)
        msh = adjmask[b][r : r + NH]
        if r:
            msh = np.roll(msh, -r, axis=1)
        in_maps.append(
            {
                "s": np.ascontiguousarray(s_roll),
                "m": np.ascontiguousarray(msh).view(fp8np),
            }
        )
    return in_maps


def _assemble(results):
    outf = np.empty((B, N, N), dtype=np.float32)
    for c in range(8):
        b, h = divmod(c, 2)
        r = h * NH
        o = results[c]["out"]
        outf[b, r : r + NH, :] = o if r == 0 else np.roll(o, r, axis=1)
    return outf


def kernel(prop_state, mask):
    from concourse.bass_utils import run_bass_kernel_spmd

    global _prog
    if _prog is None:
        _prog = _build_program()
    in_maps = _host_prep(prop_state, mask)
    res = run_bass_kernel_spmd(_prog, in_maps, core_ids=list(range(8)))
    return _assemble(res.results)



# revision 2
# speedup vs baseline: 1.7092x; 1.7092x over previous
"""Trainium2 kernel for nn_COSSIMMLP (gnn_message_passing).

reference semantics:
    src = prop_state[b, mask[...,0]]; dst = prop_state[b, mask[...,1]]
    vals = sigmoid(cossim(src, dst))          # [B, E]
    adj[b, i, j] = vals; adj[b, j, i] = vals  # dense [B, N, N]

Every scatter write at position (r, c) carries the identical value
sigmoid(cos(s_r, s_c)), so the output is exactly

    adj = sigmoid(S_hat @ S_hat.T + Madd),  Madd = 0 at edge positions,
                                                   -240 elsewhere

with S_hat the eps-clamp-normalized rows.  sigmoid(x - 240) underflows to 0,
so non-edges are (numerically exact) zero.  adj is exactly SYMMETRIC, so the
device only computes a folded half of it:

Sharding: 8 cores = 4 batches x 2 LHS-tile-halves.  Node order is rolled per
core (by 2048*h) so one SPMD program serves all cores.  In 128-row tile
coordinates (32 tiles per batch), the core owning LHS tiles m=0..15 computes
gram blocks (m, m+d) for ring distance d=0..15 (phase B, a [2048, 2048] slab
of consecutive columns in rolled order) and d=16 (phase C, 16 [128,128]
blocks, redundantly on both cores of the batch).  Together the two cores
cover every unordered tile pair; the host mirrors each off-diagonal block
into its transposed position (pure data movement).

The additive mask is an fp8 0/-240 matrix folded into the PE accumulation
group via an identity matmul.  It depends only on the integer index tensor,
so the host precomputes it; all float math (normalization, gram matmul,
sigmoid) runs on device.  Output is written f16 (quantization rel-err ~1e-4)
and upconverted to f32 during host assembly.
"""

import numpy as np
import ml_dtypes
from numpy.lib.stride_tricks import as_strided

B, N, D, E = 4, 4096, 256, 131072
P = 128              # partitions
NT = N // P          # 32 node tiles per batch
MT = 16              # LHS tiles per core (2048 rows)
ND = 16              # phase-B ring distances d=0..15 (2048 cols)
ROWS = MT * P        # 2048
COLS = ND * P        # 2048
GRP = 8              # node tiles per normalization group
EPS = 1e-8
MASK_OFF = 0xF7      # fp8_e4m3 encoding of -240.0 (sigmoid underflows to 0f)

_prog = None


def _build_program():
    import concourse.tile as tile
    from concourse import bacc, mybir
    from concourse.masks import make_identity

    f32 = mybir.dt.float32
    f16 = mybir.dt.float16
    fp8 = mybir.dt.float8e4
    ACT = mybir.ActivationFunctionType

    nc = bacc.Bacc("TRN2", target_bir_lowering=False, debug=False)
    s_in = nc.dram_tensor("s", [N, D], f32, kind="ExternalInput")
    mb_in = nc.dram_tensor("mb", [ROWS, COLS], fp8, kind="ExternalInput")
    mc_in = nc.dram_tensor("mc", [ROWS, P], fp8, kind="ExternalInput")
    outb = nc.dram_tensor("outb", [ROWS, COLS], f16, kind="ExternalOutput")
    outc = nc.dram_tensor("outc", [ROWS, P], f16, kind="ExternalOutput")

    mc_r = mc_in.rearrange("(t p) c -> p t c", p=P)
    outc_r = outc.rearrange("(t p) c -> p t c", p=P)

    with tile.TileContext(nc) as tc:
        with tc.tile_pool(name="const", bufs=1) as cpool:
            ident16 = cpool.tile([P, P], f16)
            make_identity(nc, ident16[:])
            ident8 = cpool.tile([P, P], fp8)
            make_identity(nc, ident8[:])
            # S_hat.T, split into the two 128-dim chunks of D=256
            st0 = cpool.tile([P, N], f16)
            st1 = cpool.tile([P, N], f16)
            # phase-C mask (tiny): load once up front
            mc_sb = cpool.tile([P, MT, P], fp8)
            nc.sync.dma_start(out=mc_sb[:], in_=mc_r[:, :, :])
            otc_all = cpool.tile([P, MT, P], f16)

            # ---- phase A: load, normalize (per group of 8 node tiles), transpose
            with (
                tc.tile_pool(name="prep", bufs=1) as prep,
                tc.tile_pool(name="prep_g", bufs=2) as prep_g,
                tc.tile_pool(name="prep_sc", bufs=3) as prep_sc,
                tc.tile_pool(name="prep_ps", bufs=4, space="PSUM") as prep_ps,
            ):
                s_sb = prep.tile([P, NT, D], f32)
                shat = prep.tile([P, NT, D], f16)
                s_r = s_in.rearrange("(t p) d -> p t d", p=P)
                for grp in range(NT // GRP):
                    t0 = grp * GRP
                    nc.sync.dma_start(
                        out=s_sb[:, t0 : t0 + GRP, :], in_=s_r[:, t0 : t0 + GRP, :]
                    )
                    nsq = prep_g.tile([P, GRP], f32, tag="nsq")
                    for i in range(GRP):
                        sq = prep_sc.tile([P, D], f32, tag="sq")
                        nc.scalar.activation(
                            out=sq[:], in_=s_sb[:, t0 + i, :], func=ACT.Square,
                            accum_out=nsq[:, i : i + 1],
                        )
                    nrm = prep_g.tile([P, GRP], f32, tag="nrm")
                    nc.scalar.activation(out=nrm[:], in_=nsq[:], func=ACT.Sqrt)
                    nc.vector.tensor_scalar_max(out=nrm[:], in0=nrm[:], scalar1=EPS)
                    inv = prep_g.tile([P, GRP], f32, tag="inv")
                    nc.vector.reciprocal(out=inv[:], in_=nrm[:])
                    for i in range(GRP):
                        nc.vector.tensor_scalar_mul(
                            out=shat[:, t0 + i, :],
                            in0=s_sb[:, t0 + i, :],
                            scalar1=inv[:, i : i + 1],
                        )
                    for i in range(GRP):
                        t = t0 + i
                        for dch, std in ((0, st0), (1, st1)):
                            pt = prep_ps.tile([P, P], f16, tag="tp")
                            nc.tensor.transpose(
                                pt[:], shat[:, t, dch * P : (dch + 1) * P], ident16[:]
                            )
                            nc.vector.tensor_copy(
                                out=std[:, t * P : (t + 1) * P], in_=pt[:]
                            )

            # ---- phase C: the 16 ring-distance-16 blocks (m, m+16) ----
            with tc.tile_pool(name="cps", bufs=2, space="PSUM") as cps:
                for grp in range(4):
                    psc = cps.tile([P, 4 * P], f32, tag="psc")
                    for q in range(4):
                        m = grp * 4 + q
                        for k, stk in ((0, st0), (1, st1)):
                            nc.tensor.matmul(
                                psc[:, q * P : (q + 1) * P],
                                lhsT=stk[:, m * P : (m + 1) * P],
                                rhs=stk[:, (m + 16) * P : (m + 17) * P],
                                start=(k == 0),
                                stop=False,
                            )
                        nc.tensor.matmul(
                            psc[:, q * P : (q + 1) * P],
                            lhsT=ident8[:],
                            rhs=mc_sb[:, grp * 4 + q, :],
                            start=False,
                            stop=True,
                        )
                    nc.scalar.activation(
                        out=otc_all[:, grp * 4 : grp * 4 + 4, :],
                        in_=psc[:],
                        func=ACT.Sigmoid,
                    )
                nc.sync.dma_start(out=outc_r[:, :, :], in_=otc_all[:])

            # ---- phase B: folded gram + fp8 mask add -> sigmoid -> store ----
            with (
                tc.tile_pool(name="mrow", bufs=8) as mrow,
                tc.tile_pool(name="outp", bufs=3) as outp,
                tc.tile_pool(name="mmps", bufs=2, space="PSUM") as mmps,
            ):
                for m in range(MT):
                    msk = mrow.tile([P, COLS], fp8, tag="msk")
                    nc.scalar.dma_start(out=msk[:], in_=mb_in[m * P : (m + 1) * P, :])
                    ot = outp.tile([P, COLS], f16, tag="ot")
                    ps = mmps.tile([P, COLS], f32, tag="ps")
                    base = m * P
                    for q in range(4):
                        c0, c1 = q * 512, (q + 1) * 512
                        for k, stk in ((0, st0), (1, st1)):
                            nc.tensor.matmul(
                                ps[:, c0:c1],
                                lhsT=stk[:, base : base + P],
                                rhs=stk[:, base + c0 : base + c1],
                                start=(k == 0),
                                stop=False,
                            )
                        nc.tensor.matmul(
                            ps[:, c0:c1],
                            lhsT=ident8[:],
                            rhs=msk[:, c0:c1],
                            start=False,
                            stop=True,
                        )
                    nc.scalar.activation(out=ot[:], in_=ps[:], func=ACT.Sigmoid)
                    nc.sync.dma_start(out=outb[m * P : (m + 1) * P, :], in_=ot[:])

    nc.compile()
    return nc


def _host_prep(prop_state, mask):
    prop = np.ascontiguousarray(np.asarray(prop_state), dtype=np.float32)
    mk = np.asarray(mask)
    i = mk[..., 0].astype(np.int64)
    j = mk[..., 1].astype(np.int64)
    fp8np = ml_dtypes.float8_e4m3

    in_maps = []
    for c in range(8):
        b, h = divmod(c, 2)
        r = h * ROWS
        s_roll = prop[b] if r == 0 else np.roll(prop[b], -r, axis=0)
        # adjacency mask already in this core's rolled node order
        rm = np.full((N, N), MASK_OFF, dtype=np.uint8)
        flat = rm.reshape(-1)
        i2 = (i[b] - r) % N
        j2 = (j[b] - r) % N
        flat[i2 * N + j2] = 0
        flat[j2 * N + i2] = 0
        # phase-B slabs: rows m*128..(m+1)*128, cols m*128..m*128+2048
        mb = np.ascontiguousarray(
            as_strided(rm, (MT, P, COLS), (P * (N + 1), N, 1))
        ).reshape(ROWS, COLS)
        # phase-C blocks: rows m*128..(m+1)*128, cols m*128+2048..m*128+2176
        mcs = np.ascontiguousarray(
            as_strided(rm[:, COLS:], (MT, P, P), (P * (N + 1), N, 1))
        ).reshape(ROWS, P)
        in_maps.append(
            {
                "s": np.ascontiguousarray(s_roll),
                "mb": mb.view(fp8np),
                "mc": mcs.view(fp8np),
            }
        )
    return in_maps


def _assemble(results):
    out = np.empty((B, N, N), dtype=np.float32)
    for c in range(8):
        b, h = divmod(c, 2)
        t0 = MT * h
        ob = results[c]["outb"].reshape(MT, P, ND, P)
        oc = results[c]["outc"].reshape(MT, P, P)
        for m in range(MT):
            gr = (m + t0) % NT
            rs = slice(gr * P, (gr + 1) * P)
            out[b, rs, rs] = ob[m, :, 0, :]
            for d in range(1, ND):
                gc = (m + d + t0) % NT
                cs = slice(gc * P, (gc + 1) * P)
                blk = ob[m, :, d, :]
                out[b, rs, cs] = blk
                out[b, cs, rs] = blk.T
            gc = (m + 16 + t0) % NT
            cs = slice(gc * P, (gc + 1) * P)
            blk = oc[m]
            out[b, rs, cs] = blk
            out[b, cs, rs] = blk.T
    return out


def kernel(prop_state, mask):
    from concourse.bass_utils import run_bass_kernel_spmd

    global _prog
    if _prog is None:
        _prog = _build_program()
    in_maps = _host_prep(prop_state, mask)
    res = run_bass_kernel_spmd(_prog, in_maps, core_ids=list(range(8)))
    return _assemble(res.results)
